# revision 25
# baseline (speedup 1.0000x reference)
"""DeformConv (B=8, C=256, H=W=64, O=256, 3x3, DG=1) Trainium2 Bass kernel.

Sharding: data-parallel over batch, one batch element per NeuronCore (8 cores).

Per-core pipeline (B=1):
  1. x [256,4096] f32 loaded via HWDGE (4 column-group pieces), PE-transposed
     in f32, cast to fp16 on the ACT PSUM->SBUF copy -> xt_sb [4096pos, 256ch].
     Patch table in DRAM: x_patch[lin] = [x_t[lin], x_t[lin+1], x_t[lin+64],
     x_t[lin+65]] (2KB rows) via 7 shifted strided DMA writes.
  2. Coords on DVE (f32): y0=floor(sy) (magic-number round + is_gt fix),
     base row r=clip(y0,0,62), col b=clip(x0,0,62), separable slot weights
     ws[4] reproducing mmcv zero-padding bilinear exactly (fp16 copies).
  3. Gather: one prepare_only dma_gather + trigger per (1024-pos chunk, tap):
     2KB elems from x_patch, alternating SWDGE queues; gpsimd only runs
     descriptor-gen, the drain is paced by the SDMA engines.
  4. Blend: corner products split ACT (corners 0,3: per-partition fp16 scale)
     and DVE (corners 1,2: broadcast tensor_tensor); 3 adds on DVE.
  5. PE-transpose blended [pos,ch]->[ch,pos] (fp16), PSUM->SBUF copies split
     ACT/DVE, then per-tap GEMM accumulation into 4 persistent PSUM banks
     (18 contraction blocks of 128, fp16 operands, f32 PSUM).
"""

import dataclasses

import numpy as np

_CACHE = {}

H = 64
W = 64
HW = 4096
C = 256
O = 256
K = 9
NCORES = 8
MAGIC = float(3 << 22)  # 1.5*2^23: keeps x+MAGIC in [2^23, 2^24) for |x|<2^22
USE_PREP_GATHER = False


def _step0(ap, inner):
    """Expand a [128, n] AP to [128, n, inner] with stride-0 inner dim."""
    return dataclasses.replace(ap, ap=list(ap.ap) + [[0, inner]])


def _emit(tc, nc, aps, rec=None, queue_plan=None):
    import contextlib

    import concourse.bass as bass
    import concourse.mybir as mybir
    from concourse.masks import make_identity

    dt = mybir.dt
    Alu = mybir.AluOpType
    Act = mybir.ActivationFunctionType

    x_in = aps["x"]          # [256, 4096] f32
    off_in = aps["offset"]   # [18, 4096]  f32
    w2_in = aps["w2"]        # [2304, 256] f32   (k-major, then c; lhsT layout)
    out_d = aps["out"]       # [256, 4096] f32

    ctx = contextlib.ExitStack()
    with ctx:
        # ---------------- pools ----------------
        cpool = ctx.enter_context(tc.tile_pool(name="cpool", bufs=1))
        dpool = ctx.enter_context(tc.tile_pool(name="dpool", bufs=1, space="DRAM"))

        # ---------------- persistent tiles ----------------
        ident16 = cpool.tile([128, 128], dt.float16, name="ident16")
        ident32 = cpool.tile([128, 128], dt.float32, name="ident32")
        make_identity(nc, ident16)
        make_identity(nc, ident32)

        w2_sb = cpool.tile([128, 18, 256], dt.float16, name="w2_sb")
        _i = nc.gpsimd.dma_start(
            out=w2_sb, in_=w2_in.rearrange("(kb ci) o -> ci kb o", ci=128)
        )
        if rec is not None:
            rec["plain"].append(_i.ins if hasattr(_i, "ins") else _i)
        # slot-weight fields [128 (p%128), st, K*32] and wrapped gather idx
        # ws16 holds corners 1,2 (DVE broadcast mults); ws32 holds corners
        # 0,3 in fp32 (ACT scale APs must be fp32)
        ws16 = cpool.tile([128, 4, K * 32], dt.float16, name="ws16")
        ws32 = cpool.tile([128, 2, K * 32], dt.float32, name="ws32")
        idxw = cpool.tile([128, K * 4 * 64], dt.int16, name="idxw")

        x_patch = dpool.tile([HW, 1024], dt.float16, name="x_patch")

        gsem = [nc.alloc_semaphore("gsem0"), nc.alloc_semaphore("gsem1")]

        # ================= PREP PHASE (scoped pools) =================
        with tc.tile_pool(name="prep", bufs=1) as pp, tc.tile_pool(
            name="ppsum", bufs=2, space="PSUM"
        ) as pps:
            eng = [nc.sync, nc.scalar]
            # ---- x load (f32, HWDGE, 4 pieces) ----
            x_sb = pp.tile([128, 2, HW], dt.float32, name="x_sb")
            xr = x_in.rearrange("(h c) p -> c h p", h=2)
            for piece in range(4):
                sl = slice(piece * 1024, (piece + 1) * 1024)
                eng[piece % 2].dma_start(out=x_sb[:, :, sl], in_=xr[:, :, sl])
            # ---- offsets load ----
            off_sb = pp.tile([18, HW], dt.float32, name="off_sb")
            nc.sync.dma_start(out=off_sb, in_=off_in)

            # ---- x transpose (f32) + cast-to-fp16 copies, written straight
            # into slot 0 of the interleaved patch-row tile: xt4[j, i, st, :]
            # = xt[i*128+j + sh(st)].  SBUF->SBUF shifted copies dodge the
            # HBM small-descriptor penalty; the DRAM patch write below then
            # has fully contiguous 2KB rows (4x fewer descriptors). ----
            xt4 = pp.tile([128, 32, 4, 256], dt.float16, name="xt4")
            for i in range(32):
                xtp = pps.tile([128, 256], dt.float32, name="xtp", tag="xtp")
                for h in range(2):
                    nc.tensor.transpose(
                        xtp[:, h * 128 : (h + 1) * 128],
                        x_sb[:, h, i * 128 : (i + 1) * 128],
                        ident32,
                    )
                nc.scalar.activation(xt4[:, i, 0, :], xtp, Act.Copy)
            for s in range(2):
                for t in range(2):
                    sh = 64 * s + t
                    slot = 2 * s + t
                    if sh == 0:
                        continue
                    # dst row lin = i*128+j reads xt row lin+sh:
                    # j < 128-sh -> src partition j+sh, col i
                    eng[slot % 2].dma_start(
                        out=xt4[0 : 128 - sh, :, slot, :],
                        in_=xt4[sh:128, :, 0, :],
                    )
                    # j >= 128-sh -> src partition j+sh-128, col i+1
                    eng[(slot + 1) % 2].dma_start(
                        out=xt4[128 - sh : 128, 0:31, slot, :],
                        in_=xt4[0:sh, 1:32, 0, :],
                    )
            # ---- patch table write: contiguous 2KB rows ----
            for half in range(2):
                j0 = half * 64
                dst = bass.AP(
                    tensor=x_patch.tensor,
                    offset=x_patch.offset + j0 * 1024,
                    ap=[[1024, 64], [128 * 1024, 32], [1, 1024]],
                )
                eng[half].dma_start(out=dst, in_=xt4[j0 : j0 + 64, :, :, :])

            # ---- offsets -> p-major layout via PE transpose ----
            offp = pp.tile([128, 32, 18], dt.float32, name="offp")
            for i in range(32):
                pso = pps.tile([128, 18], dt.float32, name="pso", tag="pso")
                nc.tensor.transpose(
                    pso, off_sb[:, i * 128 : (i + 1) * 128], ident32[0:18, 0:18]
                )
                nc.vector.tensor_copy(offp[:, i, :], pso)

            # ---- position iota ----
            pos_i = pp.tile([128, 32], dt.int32, name="pos_i")
            nc.gpsimd.iota(pos_i, pattern=[[128, 32]], base=0, channel_multiplier=1)
            POS = pp.tile([128, 32], dt.float32, name="POS")
            nc.vector.tensor_copy(POS, pos_i)
            Pq = pp.tile([128, 32], dt.float32, name="Pq")
            nc.vector.tensor_scalar(Pq, POS, 1.0 / 64.0, None, Alu.mult)
            I_ = pp.tile([128, 32], dt.float32, name="I_")
            CMP = pp.tile([128, 32], dt.float32, name="CMPij")
            nc.vector.tensor_scalar(CMP, Pq, MAGIC, None, Alu.add)
            nc.vector.tensor_scalar(I_, CMP, MAGIC, None, Alu.subtract)
            nc.vector.tensor_tensor(CMP, I_, Pq, Alu.is_gt)
            nc.vector.tensor_tensor(I_, I_, CMP, Alu.subtract)
            J_ = pp.tile([128, 32], dt.float32, name="J_")
            nc.vector.scalar_tensor_tensor(J_, I_, -64.0, POS, Alu.mult, Alu.add)

            # ---- per-axis coordinate pipeline ----
            KI = [k // 3 for k in range(K)]
            KJ = [k % 3 for k in range(K)]

            def axis_pipeline(off_field, base_tile, kshift, L, WS0, WS1, R_out):
                F = K * 32
                S = pp.tile([128, F], dt.float32, name=f"S{L}", tag=f"S{L}")
                for k in range(K):
                    nc.vector.scalar_tensor_tensor(
                        S[:, k * 32 : (k + 1) * 32],
                        off_field(k),
                        float(kshift[k] - 1),
                        base_tile,
                        Alu.add,
                        Alu.add,
                    )
                t = lambda nm: pp.tile([128, F], dt.float32, name=nm, tag=nm)
                Y0 = t(f"Y0{L}")
                Ct = t(f"Ct{L}")
                nc.vector.tensor_scalar(Ct, S, MAGIC, None, Alu.add)
                nc.vector.tensor_scalar(Y0, Ct, MAGIC, None, Alu.subtract)
                nc.vector.tensor_tensor(Ct, Y0, S, Alu.is_gt)
                nc.vector.tensor_tensor(Y0, Y0, Ct, Alu.subtract)
                LY = t(f"LY{L}")
                nc.vector.tensor_tensor(LY, S, Y0, Alu.subtract)
                WY0 = t(f"WY0{L}")
                nc.vector.tensor_scalar(WY0, LY, -1.0, 1.0, Alu.mult, Alu.add)
                V0 = t(f"V0{L}")
                V1 = t(f"V1{L}")
                nc.vector.tensor_scalar(V0, Y0, 0.0, None, Alu.is_ge)
                nc.vector.tensor_scalar(Ct, Y0, 63.0, None, Alu.is_le)
                nc.vector.tensor_tensor(V0, V0, Ct, Alu.mult)
                nc.vector.tensor_scalar(V1, Y0, -1.0, None, Alu.is_ge)
                nc.vector.tensor_scalar(Ct, Y0, 62.0, None, Alu.is_le)
                nc.vector.tensor_tensor(V1, V1, Ct, Alu.mult)
                nc.vector.tensor_tensor(WY0, WY0, V0, Alu.mult)
                nc.vector.tensor_tensor(LY, LY, V1, Alu.mult)
                R = R_out
                nc.vector.tensor_scalar(R, Y0, 0.0, 62.0, Alu.max, Alu.min)
                C0 = t(f"C0{L}")
                C1 = t(f"C1{L}")
                nc.vector.tensor_scalar(C0, Y0, 0.0, 63.0, Alu.max, Alu.min)
                nc.vector.tensor_scalar(C1, Y0, 1.0, 0.0, Alu.add, Alu.max)
                nc.vector.tensor_scalar(C1, C1, 63.0, None, Alu.min)
                E = t(f"E{L}")
                T1 = t(f"T1{L}")
                nc.vector.tensor_tensor(E, C0, R, Alu.is_equal)
                nc.vector.tensor_tensor(T1, WY0, E, Alu.mult)
                nc.vector.tensor_tensor(E, C1, R, Alu.is_equal)
                nc.vector.tensor_tensor(E, LY, E, Alu.mult)
                nc.vector.tensor_tensor(WS0, T1, E, Alu.add)
                Rp = t(f"Rp{L}")
                nc.vector.tensor_scalar(Rp, R, 1.0, None, Alu.add)
                nc.vector.tensor_tensor(E, C0, Rp, Alu.is_equal)
                nc.vector.tensor_tensor(T1, WY0, E, Alu.mult)
                nc.vector.tensor_tensor(E, C1, Rp, Alu.is_equal)
                nc.vector.tensor_tensor(E, LY, E, Alu.mult)
                nc.vector.tensor_tensor(WS1, T1, E, Alu.add)

            F = K * 32
            WSY0 = pp.tile([128, F], dt.float32, name="WSY0")
            WSY1 = pp.tile([128, F], dt.float32, name="WSY1")
            WSX0 = pp.tile([128, F], dt.float32, name="WSX0")
            WSX1 = pp.tile([128, F], dt.float32, name="WSX1")
            RY = pp.tile([128, F], dt.float32, name="RY")
            RX = pp.tile([128, F], dt.float32, name="RX")
            axis_pipeline(lambda k: offp[:, :, 2 * k], I_, KI, "y", WSY0, WSY1, RY)
            axis_pipeline(
                lambda k: offp[:, :, 2 * k + 1], J_, KJ, "x", WSX0, WSX1, RX
            )
            WSf = pp.tile([128, F], dt.float32, name="WSf", tag="WSf")
            nc.vector.tensor_tensor(ws32[:, 0, :], WSY0, WSX0, Alu.mult)
            nc.vector.tensor_tensor(ws32[:, 1, :], WSY1, WSX1, Alu.mult)
            nc.vector.tensor_copy(ws16[:, 3, :], ws32[:, 1, :])
            for st, (wy, wx) in [(1, (WSY0, WSX1)), (2, (WSY1, WSX0))]:
                nc.vector.tensor_tensor(WSf, wy, wx, Alu.mult)
                nc.vector.tensor_copy(ws16[:, st, :], WSf)

            # ---- gather indices: lin = RY*64 + RX, cast to i16 ----
            IDX = pp.tile([128, 384], dt.float32, name="IDX")
            nc.gpsimd.memset(IDX, 0)
            nc.vector.scalar_tensor_tensor(
                IDX[:, 0:F], RY, 64.0, RX, Alu.mult, Alu.add
            )
            # shuffle p%128 -> p%16 wrap via two PE transpose stages (f32),
            # casting to i16 on the final PSUM->SBUF copy:
            # idxw[t, (k,ch)*64 + bl*8 + g] = IDX[g*16+t, k*32+ch*8+bl]
            t1sb = pp.tile([128, 3, 128], dt.float32, name="t1sb")
            for ct in range(3):
                ps1 = pps.tile([128, 128], dt.float32, name="ps1", tag="ps1")
                nc.tensor.transpose(ps1, IDX[:, ct * 128 : (ct + 1) * 128], ident32)
                nc.vector.tensor_copy(t1sb[:, ct, :], ps1)
            # stage 2: per (ct, g): [128col, 16] -> [16, 128col]
            for ct in range(3):
                nk = 4 if ct < 2 else 1  # k-count covered by this col tile
                for g in range(8):
                    ps2 = pps.tile([16, 128], dt.float32, name="ps2", tag="ps2")
                    nc.tensor.transpose(
                        ps2, t1sb[:, ct, g * 16 : (g + 1) * 16], ident32
                    )
                    # dst cols: for k' in [0,nk), ch in 4, bl in 8:
                    #   ((ct*4+k')*4+ch)*64 + bl*8 + g
                    dst = bass.AP(
                        tensor=idxw.tensor,
                        offset=idxw.offset + (ct * 4 * 4) * 64 + g,
                        ap=[[idxw.ap[0][0], 16], [256, nk], [64, 4], [8, 8]],
                    )
                    nc.vector.tensor_copy(
                        dst,
                        ps2[0:16, 0 : nk * 32].rearrange(
                            "t (k c b) -> t k c b", k=nk, c=4
                        ),
                    )
            # idxw replication on SWDGE: keeps it off the HWDGE rings where
            # it would queue behind the patch-table writes (FIFO per ring).
            # Not recorded as plain-lane DMAs: every gather depends on idxw,
            # so these complete strictly before any gather fires and cannot
            # invert tick order on a shared DMASW lane.
            for rep in range(1, 8):
                nc.gpsimd.dma_start(
                    out=idxw[rep * 16 : (rep + 1) * 16, :], in_=idxw[0:16, :]
                )

        # ================= MAIN LOOP =================
        pgpool = ctx.enter_context(tc.tile_pool(name="pgpool", bufs=1, space="PSUM"))
        ptpool = ctx.enter_context(tc.tile_pool(name="ptpool", bufs=3, space="PSUM"))
        gpool = ctx.enter_context(tc.tile_pool(name="gpool", bufs=3))
        spool = ctx.enter_context(tc.tile_pool(name="spool", bufs=2))
        bpool = ctx.enter_context(tc.tile_pool(name="bpool", bufs=2))
        opool = ctx.enter_context(tc.tile_pool(name="opool", bufs=2))

        for ch in range(4):  # 1024-position chunks
            pg = [
                pgpool.tile([128, 512], dt.float32, name=f"pg{ms}", tag=f"pg{ms}")
                for ms in range(4)
            ]
            for k in range(K):
                G = gpool.tile([128, 8, 1024], dt.float16, name="G", tag="G")
                qi = ch * K + k
                q = 0 if queue_plan is None else queue_plan[qi]
                if USE_PREP_GATHER:
                    _i = nc.gpsimd.dma_gather(
                        G,
                        x_patch,
                        idxw[:, (k * 4 + ch) * 64 : (k * 4 + ch + 1) * 64],
                        num_idxs=1024,
                        num_idxs_reg=1024,
                        elem_size=1024,
                        elem_step=1024,
                        queue_num=q,
                        prepare_only=True,
                        sem=gsem[q],
                    )
                    nc.gpsimd.trigger_dma(count=1, queue_num=q)
                else:
                    _i = nc.gpsimd.dma_gather(
                        G,
                        x_patch,
                        idxw[:, (k * 4 + ch) * 64 : (k * 4 + ch + 1) * 64],
                        num_idxs=1024,
                        num_idxs_reg=1024,
                        elem_size=1024,
                        elem_step=1024,
                        queue_num=q,
                    )
                if rec is not None:
                    rec["gather"].append(_i.ins if hasattr(_i, "ins") else _i)
                # blend 4 corners: A = sum_st ws_st * G[:, :, st].
                # corners 0,3 on ACT (per-partition fp16 scale, per-bl ops);
                # corners 1,2 on DVE as fused broadcast-mults (step-0 in1).
                # A accumulates corners 0..2 + low half of corner 3; the high
                # half of corner 3 lands in P3 and is summed into the PSUM
                # transpose (accumulating matmul) instead of a DVE add.
                A = bpool.tile([128, 8, 256], dt.float16, name="A", tag="A")
                Mt = bpool.tile([128, 8, 256], dt.float16, name="Mt", tag="Mt")
                P0 = bpool.tile([128, 8, 256], dt.float16, name="P0", tag="P0")
                P3 = bpool.tile([128, 8, 256], dt.float16, name="P3", tag="P3")
                # ACT computes P3 (6 blocks) first so the DVE add-chain can
                # consume it early; P0 lands last and is added last.
                for bl in range(6):
                    wc = k * 32 + ch * 8 + bl
                    nc.scalar.activation(
                        P3[:, bl, :],
                        G[:, bl, 768:1024],
                        Act.Copy,
                        scale=ws32[:, 1, wc : wc + 1],
                    )
                for bl in range(8):
                    wc = k * 32 + ch * 8 + bl
                    nc.scalar.activation(
                        P0[:, bl, :],
                        G[:, bl, 0:256],
                        Act.Copy,
                        scale=ws32[:, 0, wc : wc + 1],
                    )
                wsl = lambda st: ws16[:, st, k * 32 + ch * 8 : k * 32 + (ch + 1) * 8]
                nc.vector.tensor_tensor(
                    P3[:, 6:8, :],
                    G[:, 6:8, 768:1024],
                    _step0(wsl(3)[:, 6:8], 256),
                    Alu.mult,
                )
                nc.vector.tensor_tensor(
                    A, G[:, :, 256:512], _step0(wsl(1), 256), Alu.mult
                )
                nc.vector.tensor_tensor(A, A, P3, Alu.add)
                nc.vector.tensor_tensor(
                    Mt, G[:, :, 512:768], _step0(wsl(2), 256), Alu.mult
                )
                nc.vector.tensor_tensor(A, A, Mt, Alu.add)
                nc.vector.tensor_tensor(A, A, P0, Alu.add)
                # transpose [pos, ch] -> [ch, pos], accumulating A + P3 in
                # PSUM; PSUM->SBUF copies split ACT (h=0) / DVE (h=1); then
                # per-tap GEMM accumulation.
                Ssb = spool.tile([128, 2, 1024], dt.float16, name="Ssb", tag="Ssb")
                for h in range(2):
                    for blq in range(2):
                        pt = ptpool.tile(
                            [128, 512], dt.float16, name="pt", tag="pt"
                        )
                        for bb in range(4):
                            bl = blq * 4 + bb
                            nc.tensor.transpose(
                                pt[:, bb * 128 : (bb + 1) * 128],
                                A[:, bl, h * 128 : (h + 1) * 128],
                                ident16,
                            )
                        dst = Ssb[:, h, blq * 512 : (blq + 1) * 512]
                        if h == 0:
                            nc.scalar.activation(dst, pt, Act.Copy)
                        else:
                            nc.vector.tensor_copy(dst, pt)
                for h in range(2):
                    kb = 2 * k + h
                    for m in range(2):
                        for sub in range(2):
                            nc.tensor.matmul(
                                pg[2 * m + sub],
                                lhsT=w2_sb[:, kb, m * 128 : (m + 1) * 128],
                                rhs=Ssb[:, h, sub * 512 : (sub + 1) * 512],
                                start=(kb == 0),
                                stop=(kb == 17),
                            )
            # PSUM evict + output store for this chunk
            for m in range(2):
                for sub in range(2):
                    ot = opool.tile([128, 512], dt.float32, name="ot", tag="ot")
                    nc.vector.tensor_copy(ot, pg[2 * m + sub])
                    nc.sync.dma_start(
                        out=out_d[
                            m * 128 : (m + 1) * 128,
                            ch * 1024 + sub * 512 : ch * 1024 + (sub + 1) * 512,
                        ],
                        in_=ot,
                    )


def _lane_of(inst):
    from concourse.tile_sem_assignment import PROC_NAME_TO_IDX

    rev = {v: k for k, v in PROC_NAME_TO_IDX.items()}
    nm = rev.get(inst.bass_scheduled_proc, "")
    return int(nm[5:]) if nm.startswith("DMASW") else None


def build(queue_plan="auto"):
    import concourse.mybir as mybir
    from concourse import bacc, tile

    dt = mybir.dt
    nc = bacc.Bacc(
        "TRN2",
        target_bir_lowering=False,
        debug=False,
        enable_asserts=False,
        num_devices=NCORES,
        num_swdge_queues=2,
    )
    aps = {
        "x": nc.dram_tensor("x", [C, HW], dt.float32, kind="ExternalInput").ap(),
        "offset": nc.dram_tensor(
            "offset", [2 * K, HW], dt.float32, kind="ExternalInput"
        ).ap(),
        "w2": nc.dram_tensor(
            "w2", [C * K, O], dt.float32, kind="ExternalInput"
        ).ap(),
        "out": nc.dram_tensor(
            "out", [O, HW], dt.float32, kind="ExternalOutput"
        ).ap(),
    }
    if queue_plan == "auto":
        # pass 1: discover each SWDGE DMA's DMASW lane, then rebuild with a
        # lane-consistent queue assignment (lane%2, forced 0 on lanes that
        # host plain queue-0 dma_starts).
        rec = {"gather": [], "plain": []}
        with tile.TileContext(nc) as tc:
            _emit(tc, nc, aps, rec=rec, queue_plan=None)
        plain_lanes = {_lane_of(i) for i in rec["plain"]}
        plan = []
        for gi in rec["gather"]:
            lane = _lane_of(gi)
            q = 0 if (lane is None or lane in plain_lanes) else lane % 2
            plan.append(q)
        return build(plan)
    with tile.TileContext(nc) as tc:
        _emit(tc, nc, aps, queue_plan=queue_plan)
    nc.compile()
    return nc


def prep_in_maps(x, offset, weight):
    x = np.asarray(x, dtype=np.float32)
    offset = np.asarray(offset, dtype=np.float32)
    weight = np.asarray(weight, dtype=np.float32)
    w2 = np.ascontiguousarray(
        weight.reshape(O, C, K).transpose(2, 1, 0).reshape(C * K, O)
    )
    in_maps = []
    for b in range(NCORES):
        in_maps.append(
            {
                "x": np.ascontiguousarray(x[b].reshape(C, HW)),
                "offset": np.ascontiguousarray(offset[b].reshape(2 * K, HW)),
                "w2": w2,
            }
        )
    return in_maps


def run(x, offset, weight, trace=False, **kw):
    from concourse import bass_utils

    if "nc" not in _CACHE:
        _CACHE["nc"] = build()
    nc = _CACHE["nc"]
    res = bass_utils.run_bass_kernel_spmd(
        nc, prep_in_maps(x, offset, weight), core_ids=list(range(NCORES)),
        trace=trace, **kw,
    )
    out = np.stack([r["out"].reshape(O, H, W) for r in res.results])
    return out, res


def kernel(x, offset, weight):
    out, _ = run(x, offset, weight, trace=False)
    return out


# revision 28
# speedup vs baseline: 1.2122x; 1.2122x over previous
"""DeformConv (B=8, C=256, H=W=64, O=256, 3x3, DG=1) Trainium2 Bass kernel.

Sharding: data-parallel over batch, one batch element per NeuronCore (8 cores).

Per-core pipeline (B=1):
  1. x [256,4096] f32 loaded via HWDGE (4 column-group pieces), PE-transposed
     in f32, cast to fp16 on the ACT PSUM->SBUF copy -> xt_sb [4096pos, 256ch].
     Patch table in DRAM: x_patch[lin] = [x_t[lin], x_t[lin+1], x_t[lin+64],
     x_t[lin+65]] (2KB rows) via 7 shifted strided DMA writes.
  2. Coords on DVE (f32): y0=floor(sy) (magic-number round + is_gt fix),
     base row r=clip(y0,0,62), col b=clip(x0,0,62), separable slot weights
     ws[4] reproducing mmcv zero-padding bilinear exactly (fp16 copies).
  3. Gather: one prepare_only dma_gather + trigger per (1024-pos chunk, tap):
     2KB elems from x_patch, alternating SWDGE queues; gpsimd only runs
     descriptor-gen, the drain is paced by the SDMA engines.
  4. Blend: corner products split ACT (corners 0,3: per-partition fp16 scale)
     and DVE (corners 1,2: broadcast tensor_tensor); 3 adds on DVE.
  5. PE-transpose blended [pos,ch]->[ch,pos] (fp16), PSUM->SBUF copies split
     ACT/DVE, then per-tap GEMM accumulation into 4 persistent PSUM banks
     (18 contraction blocks of 128, fp16 operands, f32 PSUM).
"""

import dataclasses

import numpy as np

_CACHE = {}

H = 64
W = 64
HW = 4096
C = 256
O = 256
K = 9
NCORES = 8
MAGIC = float(3 << 22)  # 1.5*2^23: keeps x+MAGIC in [2^23, 2^24) for |x|<2^22
USE_PREP_GATHER = False


def _step0(ap, inner):
    """Expand a [128, n] AP to [128, n, inner] with stride-0 inner dim."""
    return dataclasses.replace(ap, ap=list(ap.ap) + [[0, inner]])


def _emit(tc, nc, aps, rec=None, queue_plan=None):
    import contextlib

    import concourse.bass as bass
    import concourse.mybir as mybir
    from concourse.masks import make_identity

    dt = mybir.dt
    Alu = mybir.AluOpType
    Act = mybir.ActivationFunctionType

    x_in = aps["x"]          # [256, 4096] f32
    off_in = aps["offset"]   # [18, 4096]  f32
    w2_in = aps["w2"]        # [2304, 256] f32   (k-major, then c; lhsT layout)
    out_d = aps["out"]       # [256, 4096] f32

    ctx = contextlib.ExitStack()
    with ctx:
        # ---------------- pools ----------------
        cpool = ctx.enter_context(tc.tile_pool(name="cpool", bufs=1))
        dpool = ctx.enter_context(tc.tile_pool(name="dpool", bufs=1, space="DRAM"))

        # ---------------- persistent tiles ----------------
        ident16 = cpool.tile([128, 128], dt.float16, name="ident16")
        ident32 = cpool.tile([128, 128], dt.float32, name="ident32")
        make_identity(nc, ident16)
        make_identity(nc, ident32)

        w2_sb = cpool.tile([128, 18, 256], dt.float16, name="w2_sb")
        _i = nc.gpsimd.dma_start(
            out=w2_sb, in_=w2_in.rearrange("(kb ci) o -> ci kb o", ci=128)
        )
        if rec is not None:
            rec["plain"].append(_i.ins if hasattr(_i, "ins") else _i)
        # slot-weight fields [128 (p%128), st, K*32] and wrapped gather idx
        # ws16 holds corners 1,2 (DVE broadcast mults); ws32 holds corners
        # 0,3 in fp32 (ACT scale APs must be fp32)
        ws16 = cpool.tile([128, 4, K * 32], dt.float16, name="ws16")
        ws32 = cpool.tile([128, 2, K * 32], dt.float32, name="ws32")
        idxw = cpool.tile([128, K * 4 * 64], dt.int16, name="idxw")

        x_patch = dpool.tile([HW, 1024], dt.float16, name="x_patch")

        gsem = [nc.alloc_semaphore("gsem0"), nc.alloc_semaphore("gsem1")]

        # ================= PREP PHASE (scoped pools) =================
        with tc.tile_pool(name="prep", bufs=1) as pp, tc.tile_pool(
            name="ppsum", bufs=2, space="PSUM"
        ) as pps:
            eng = [nc.sync, nc.scalar]
            # ---- x load (f32, HWDGE, 4 pieces) ----
            x_sb = pp.tile([128, 2, HW], dt.float32, name="x_sb")
            xr = x_in.rearrange("(h c) p -> c h p", h=2)
            for piece in range(4):
                sl = slice(piece * 1024, (piece + 1) * 1024)
                eng[piece % 2].dma_start(out=x_sb[:, :, sl], in_=xr[:, :, sl])
            # ---- offsets load ----
            off_sb = pp.tile([18, HW], dt.float32, name="off_sb")
            nc.sync.dma_start(out=off_sb, in_=off_in)

            # ---- x transpose (f32) + cast-to-fp16 copies ----
            xt_sb = pp.tile([128, 32, C], dt.float16, name="xt_sb")
            for i in range(32):
                xtp = pps.tile([128, 256], dt.float32, name="xtp", tag="xtp")
                for h in range(2):
                    nc.tensor.transpose(
                        xtp[:, h * 128 : (h + 1) * 128],
                        x_sb[:, h, i * 128 : (i + 1) * 128],
                        ident32,
                    )
                nc.scalar.activation(xt_sb[:, i, :], xtp, Act.Copy)
            # ---- patch table: x_patch[lin, (s,t)*256:+256] = xt[lin+64s+t] ----
            # writes split between the two HWDGE engines (sync + scalar)
            for s in range(2):
                for t in range(2):
                    sh = 64 * s + t
                    slot = (2 * s + t) * 256
                    # rows p = i*128+j ; dst row p-sh for p >= sh
                    for half in range(2):
                        i0 = half * 16
                        dst_a = bass.AP(
                            tensor=x_patch.tensor,
                            offset=x_patch.offset + slot + i0 * 128 * 1024,
                            ap=[[1024, 128 - sh], [128 * 1024, 16], [1, 256]],
                        )
                        eng[(2 * s + t + half) % 2].dma_start(
                            out=dst_a, in_=xt_sb[sh:128, i0 : i0 + 16, :]
                        )
                    if sh:
                        dst_b = bass.AP(
                            tensor=x_patch.tensor,
                            offset=x_patch.offset + slot + (128 - sh) * 1024,
                            ap=[[1024, sh], [128 * 1024, 31], [1, 256]],
                        )
                        eng[(s + t) % 2].dma_start(
                            out=dst_b, in_=xt_sb[0:sh, 1:32, :]
                        )

            # ---- offsets -> p-major layout via PE transpose ----
            offp = pp.tile([128, 32, 18], dt.float32, name="offp")
            for i in range(32):
                pso = pps.tile([128, 18], dt.float32, name="pso", tag="pso")
                nc.tensor.transpose(
                    pso, off_sb[:, i * 128 : (i + 1) * 128], ident32[0:18, 0:18]
                )
                nc.vector.tensor_copy(offp[:, i, :], pso)

            # ---- position iota ----
            pos_i = pp.tile([128, 32], dt.int32, name="pos_i")
            nc.gpsimd.iota(pos_i, pattern=[[128, 32]], base=0, channel_multiplier=1)
            POS = pp.tile([128, 32], dt.float32, name="POS")
            nc.vector.tensor_copy(POS, pos_i)
            Pq = pp.tile([128, 32], dt.float32, name="Pq")
            nc.vector.tensor_scalar(Pq, POS, 1.0 / 64.0, None, Alu.mult)
            I_ = pp.tile([128, 32], dt.float32, name="I_")
            CMP = pp.tile([128, 32], dt.float32, name="CMPij")
            nc.vector.tensor_scalar(CMP, Pq, MAGIC, None, Alu.add)
            nc.vector.tensor_scalar(I_, CMP, MAGIC, None, Alu.subtract)
            nc.vector.tensor_tensor(CMP, I_, Pq, Alu.is_gt)
            nc.vector.tensor_tensor(I_, I_, CMP, Alu.subtract)
            J_ = pp.tile([128, 32], dt.float32, name="J_")
            nc.vector.scalar_tensor_tensor(J_, I_, -64.0, POS, Alu.mult, Alu.add)

            # ---- per-axis coordinate pipeline ----
            KI = [k // 3 for k in range(K)]
            KJ = [k % 3 for k in range(K)]

            def axis_pipeline(off_field, base_tile, kshift, L, WS0, WS1, R_out):
                F = K * 32
                S = pp.tile([128, F], dt.float32, name=f"S{L}", tag=f"S{L}")
                for k in range(K):
                    nc.vector.scalar_tensor_tensor(
                        S[:, k * 32 : (k + 1) * 32],
                        off_field(k),
                        float(kshift[k] - 1),
                        base_tile,
                        Alu.add,
                        Alu.add,
                    )
                t = lambda nm: pp.tile([128, F], dt.float32, name=nm, tag=nm)
                Y0 = t(f"Y0{L}")
                Ct = t(f"Ct{L}")
                nc.vector.tensor_scalar(Ct, S, MAGIC, None, Alu.add)
                nc.vector.tensor_scalar(Y0, Ct, MAGIC, None, Alu.subtract)
                nc.vector.tensor_tensor(Ct, Y0, S, Alu.is_gt)
                nc.vector.tensor_tensor(Y0, Y0, Ct, Alu.subtract)
                LY = t(f"LY{L}")
                nc.vector.tensor_tensor(LY, S, Y0, Alu.subtract)
                WY0 = t(f"WY0{L}")
                nc.vector.tensor_scalar(WY0, LY, -1.0, 1.0, Alu.mult, Alu.add)
                V0 = t(f"V0{L}")
                V1 = t(f"V1{L}")
                nc.vector.tensor_scalar(V0, Y0, 0.0, None, Alu.is_ge)
                nc.vector.tensor_scalar(Ct, Y0, 63.0, None, Alu.is_le)
                nc.vector.tensor_tensor(V0, V0, Ct, Alu.mult)
                nc.vector.tensor_scalar(V1, Y0, -1.0, None, Alu.is_ge)
                nc.vector.tensor_scalar(Ct, Y0, 62.0, None, Alu.is_le)
                nc.vector.tensor_tensor(V1, V1, Ct, Alu.mult)
                nc.vector.tensor_tensor(WY0, WY0, V0, Alu.mult)
                nc.vector.tensor_tensor(LY, LY, V1, Alu.mult)
                R = R_out
                nc.vector.tensor_scalar(R, Y0, 0.0, 62.0, Alu.max, Alu.min)
                C0 = t(f"C0{L}")
                C1 = t(f"C1{L}")
                nc.vector.tensor_scalar(C0, Y0, 0.0, 63.0, Alu.max, Alu.min)
                nc.vector.tensor_scalar(C1, Y0, 1.0, 0.0, Alu.add, Alu.max)
                nc.vector.tensor_scalar(C1, C1, 63.0, None, Alu.min)
                E = t(f"E{L}")
                T1 = t(f"T1{L}")
                nc.vector.tensor_tensor(E, C0, R, Alu.is_equal)
                nc.vector.tensor_tensor(T1, WY0, E, Alu.mult)
                nc.vector.tensor_tensor(E, C1, R, Alu.is_equal)
                nc.vector.tensor_tensor(E, LY, E, Alu.mult)
                nc.vector.tensor_tensor(WS0, T1, E, Alu.add)
                Rp = t(f"Rp{L}")
                nc.vector.tensor_scalar(Rp, R, 1.0, None, Alu.add)
                nc.vector.tensor_tensor(E, C0, Rp, Alu.is_equal)
                nc.vector.tensor_tensor(T1, WY0, E, Alu.mult)
                nc.vector.tensor_tensor(E, C1, Rp, Alu.is_equal)
                nc.vector.tensor_tensor(E, LY, E, Alu.mult)
                nc.vector.tensor_tensor(WS1, T1, E, Alu.add)

            F = K * 32
            WSY0 = pp.tile([128, F], dt.float32, name="WSY0")
            WSY1 = pp.tile([128, F], dt.float32, name="WSY1")
            WSX0 = pp.tile([128, F], dt.float32, name="WSX0")
            WSX1 = pp.tile([128, F], dt.float32, name="WSX1")
            RY = pp.tile([128, F], dt.float32, name="RY")
            RX = pp.tile([128, F], dt.float32, name="RX")
            axis_pipeline(lambda k: offp[:, :, 2 * k], I_, KI, "y", WSY0, WSY1, RY)
            axis_pipeline(
                lambda k: offp[:, :, 2 * k + 1], J_, KJ, "x", WSX0, WSX1, RX
            )
            WSf = pp.tile([128, F], dt.float32, name="WSf", tag="WSf")
            nc.vector.tensor_tensor(ws32[:, 0, :], WSY0, WSX0, Alu.mult)
            nc.vector.tensor_tensor(ws32[:, 1, :], WSY1, WSX1, Alu.mult)
            nc.vector.tensor_copy(ws16[:, 3, :], ws32[:, 1, :])
            for st, (wy, wx) in [(1, (WSY0, WSX1)), (2, (WSY1, WSX0))]:
                nc.vector.tensor_tensor(WSf, wy, wx, Alu.mult)
                nc.vector.tensor_copy(ws16[:, st, :], WSf)

            # ---- gather indices: lin = RY*64 + RX, cast to i16 ----
            IDX = pp.tile([128, 384], dt.float32, name="IDX")
            nc.gpsimd.memset(IDX, 0)
            nc.vector.scalar_tensor_tensor(
                IDX[:, 0:F], RY, 64.0, RX, Alu.mult, Alu.add
            )
            # shuffle p%128 -> p%16 wrap via two PE transpose stages (f32),
            # casting to i16 on the final PSUM->SBUF copy:
            # idxw[t, (k,ch)*64 + bl*8 + g] = IDX[g*16+t, k*32+ch*8+bl]
            t1sb = pp.tile([128, 3, 128], dt.float32, name="t1sb")
            for ct in range(3):
                ps1 = pps.tile([128, 128], dt.float32, name="ps1", tag="ps1")
                nc.tensor.transpose(ps1, IDX[:, ct * 128 : (ct + 1) * 128], ident32)
                nc.vector.tensor_copy(t1sb[:, ct, :], ps1)
            # stage 2: per (ct, g): [128col, 16] -> [16, 128col]
            for ct in range(3):
                nk = 4 if ct < 2 else 1  # k-count covered by this col tile
                for g in range(8):
                    ps2 = pps.tile([16, 128], dt.float32, name="ps2", tag="ps2")
                    nc.tensor.transpose(
                        ps2, t1sb[:, ct, g * 16 : (g + 1) * 16], ident32
                    )
                    # dst cols: for k' in [0,nk), ch in 4, bl in 8:
                    #   ((ct*4+k')*4+ch)*64 + bl*8 + g
                    dst = bass.AP(
                        tensor=idxw.tensor,
                        offset=idxw.offset + (ct * 4 * 4) * 64 + g,
                        ap=[[idxw.ap[0][0], 16], [256, nk], [64, 4], [8, 8]],
                    )
                    nc.vector.tensor_copy(
                        dst,
                        ps2[0:16, 0 : nk * 32].rearrange(
                            "t (k c b) -> t k c b", k=nk, c=4
                        ),
                    )
            for rep in range(1, 8):
                eng[rep % 2].dma_start(
                    out=idxw[rep * 16 : (rep + 1) * 16, :], in_=idxw[0:16, :]
                )

        # ================= MAIN LOOP =================
        pgpool = ctx.enter_context(tc.tile_pool(name="pgpool", bufs=1, space="PSUM"))
        ptpool = ctx.enter_context(tc.tile_pool(name="ptpool", bufs=3, space="PSUM"))
        gpool = ctx.enter_context(tc.tile_pool(name="gpool", bufs=3))
        spool = ctx.enter_context(tc.tile_pool(name="spool", bufs=2))
        bpool = ctx.enter_context(tc.tile_pool(name="bpool", bufs=2))
        opool = ctx.enter_context(tc.tile_pool(name="opool", bufs=2))

        for ch in range(4):  # 1024-position chunks
            pg = [
                pgpool.tile([128, 512], dt.float32, name=f"pg{ms}", tag=f"pg{ms}")
                for ms in range(4)
            ]
            for k in range(K):
                G = gpool.tile([128, 8, 1024], dt.float16, name="G", tag="G")
                qi = ch * K + k
                q = 0 if queue_plan is None else queue_plan[qi]
                if USE_PREP_GATHER:
                    _i = nc.gpsimd.dma_gather(
                        G,
                        x_patch,
                        idxw[:, (k * 4 + ch) * 64 : (k * 4 + ch + 1) * 64],
                        num_idxs=1024,
                        num_idxs_reg=1024,
                        elem_size=1024,
                        elem_step=1024,
                        queue_num=q,
                        prepare_only=True,
                        sem=gsem[q],
                    )
                    nc.gpsimd.trigger_dma(count=1, queue_num=q)
                else:
                    _i = nc.gpsimd.dma_gather(
                        G,
                        x_patch,
                        idxw[:, (k * 4 + ch) * 64 : (k * 4 + ch + 1) * 64],
                        num_idxs=1024,
                        num_idxs_reg=1024,
                        elem_size=1024,
                        elem_step=1024,
                        queue_num=q,
                    )
                if rec is not None:
                    rec["gather"].append(_i.ins if hasattr(_i, "ins") else _i)
                # blend 4 corners: A = sum_st ws_st * G[:, :, st].
                # corners 0,3 on ACT (per-partition fp16 scale, per-bl ops);
                # corners 1,2 on DVE as fused broadcast-mults (step-0 in1).
                # A accumulates corners 0..2 + low half of corner 3; the high
                # half of corner 3 lands in P3 and is summed into the PSUM
                # transpose (accumulating matmul) instead of a DVE add.
                A = bpool.tile([128, 8, 256], dt.float16, name="A", tag="A")
                Mt = bpool.tile([128, 8, 256], dt.float16, name="Mt", tag="Mt")
                P0 = bpool.tile([128, 8, 256], dt.float16, name="P0", tag="P0")
                P3 = bpool.tile([128, 8, 256], dt.float16, name="P3", tag="P3")
                for bl in range(8):
                    wc = k * 32 + ch * 8 + bl
                    nc.scalar.activation(
                        P0[:, bl, :],
                        G[:, bl, 0:256],
                        Act.Copy,
                        scale=ws32[:, 0, wc : wc + 1],
                    )
                    nc.scalar.activation(
                        P3[:, bl, :],
                        G[:, bl, 768:1024],
                        Act.Copy,
                        scale=ws32[:, 1, wc : wc + 1],
                    )
                wsl = lambda st: ws16[:, st, k * 32 + ch * 8 : k * 32 + (ch + 1) * 8]
                nc.vector.tensor_tensor(
                    A, G[:, :, 256:512], _step0(wsl(1), 256), Alu.mult
                )
                nc.vector.tensor_tensor(A, A, P0, Alu.add)
                nc.vector.tensor_tensor(
                    Mt, G[:, :, 512:768], _step0(wsl(2), 256), Alu.mult
                )
                nc.vector.tensor_tensor(A, A, Mt, Alu.add)
                nc.vector.tensor_tensor(A, A, P3, Alu.add)
                # transpose [pos, ch] -> [ch, pos], accumulating A + P3 in
                # PSUM; PSUM->SBUF copies split ACT (h=0) / DVE (h=1); then
                # per-tap GEMM accumulation.
                Ssb = spool.tile([128, 2, 1024], dt.float16, name="Ssb", tag="Ssb")
                for h in range(2):
                    for blq in range(2):
                        pt = ptpool.tile(
                            [128, 512], dt.float16, name="pt", tag="pt"
                        )
                        for bb in range(4):
                            bl = blq * 4 + bb
                            nc.tensor.transpose(
                                pt[:, bb * 128 : (bb + 1) * 128],
                                A[:, bl, h * 128 : (h + 1) * 128],
                                ident16,
                            )
                        dst = Ssb[:, h, blq * 512 : (blq + 1) * 512]
                        if h == 0:
                            nc.scalar.activation(dst, pt, Act.Copy)
                        else:
                            nc.vector.tensor_copy(dst, pt)
                for h in range(2):
                    kb = 2 * k + h
                    for m in range(2):
                        for sub in range(2):
                            nc.tensor.matmul(
                                pg[2 * m + sub],
                                lhsT=w2_sb[:, kb, m * 128 : (m + 1) * 128],
                                rhs=Ssb[:, h, sub * 512 : (sub + 1) * 512],
                                start=(kb == 0),
                                stop=(kb == 17),
                            )
            # PSUM evict + output store for this chunk
            for m in range(2):
                for sub in range(2):
                    ot = opool.tile([128, 512], dt.float32, name="ot", tag="ot")
                    nc.vector.tensor_copy(ot, pg[2 * m + sub])
                    nc.sync.dma_start(
                        out=out_d[
                            m * 128 : (m + 1) * 128,
                            ch * 1024 + sub * 512 : ch * 1024 + (sub + 1) * 512,
                        ],
                        in_=ot,
                    )


def _lane_of(inst):
    from concourse.tile_sem_assignment import PROC_NAME_TO_IDX

    rev = {v: k for k, v in PROC_NAME_TO_IDX.items()}
    nm = rev.get(inst.bass_scheduled_proc, "")
    return int(nm[5:]) if nm.startswith("DMASW") else None


def build(queue_plan="auto"):
    import concourse.mybir as mybir
    from concourse import bacc, tile

    dt = mybir.dt
    nc = bacc.Bacc(
        "TRN2",
        target_bir_lowering=False,
        debug=False,
        enable_asserts=False,
        num_devices=NCORES,
        num_swdge_queues=2,
    )
    aps = {
        "x": nc.dram_tensor("x", [C, HW], dt.float32, kind="ExternalInput").ap(),
        "offset": nc.dram_tensor(
            "offset", [2 * K, HW], dt.float32, kind="ExternalInput"
        ).ap(),
        "w2": nc.dram_tensor(
            "w2", [C * K, O], dt.float32, kind="ExternalInput"
        ).ap(),
        "out": nc.dram_tensor(
            "out", [O, HW], dt.float32, kind="ExternalOutput"
        ).ap(),
    }
    if queue_plan == "auto":
        # pass 1: discover each SWDGE DMA's DMASW lane, then rebuild with a
        # lane-consistent queue assignment (lane%2, forced 0 on lanes that
        # host plain queue-0 dma_starts).
        rec = {"gather": [], "plain": []}
        with tile.TileContext(nc) as tc:
            _emit(tc, nc, aps, rec=rec, queue_plan=None)
        plain_lanes = {_lane_of(i) for i in rec["plain"]}
        plan = []
        for gi in rec["gather"]:
            lane = _lane_of(gi)
            q = 0 if (lane is None or lane in plain_lanes) else lane % 2
            plan.append(q)
        return build(plan)
    with tile.TileContext(nc) as tc:
        _emit(tc, nc, aps, queue_plan=queue_plan)
    nc.compile()
    return nc


def prep_in_maps(x, offset, weight):
    x = np.asarray(x, dtype=np.float32)
    offset = np.asarray(offset, dtype=np.float32)
    weight = np.asarray(weight, dtype=np.float32)
    w2 = np.ascontiguousarray(
        weight.reshape(O, C, K).transpose(2, 1, 0).reshape(C * K, O)
    )
    in_maps = []
    for b in range(NCORES):
        in_maps.append(
            {
                "x": np.ascontiguousarray(x[b].reshape(C, HW)),
                "offset": np.ascontiguousarray(offset[b].reshape(2 * K, HW)),
                "w2": w2,
            }
        )
    return in_maps


def run(x, offset, weight, trace=False, **kw):
    from concourse import bass_utils

    if "nc" not in _CACHE:
        _CACHE["nc"] = build()
    nc = _CACHE["nc"]
    res = bass_utils.run_bass_kernel_spmd(
        nc, prep_in_maps(x, offset, weight), core_ids=list(range(NCORES)),
        trace=trace, **kw,
    )
    out = np.stack([r["out"].reshape(O, H, W) for r in res.results])
    return out, res


def kernel(x, offset, weight):
    out, _ = run(x, offset, weight, trace=False)
    return out


# revision 29
# speedup vs baseline: 1.2183x; 1.0051x over previous
"""DeformConv (B=8, C=256, H=W=64, O=256, 3x3, DG=1) Trainium2 Bass kernel.

Sharding: data-parallel over batch, one batch element per NeuronCore (8 cores).

Per-core pipeline (B=1):
  1. x [256,4096] f32 loaded via HWDGE (4 column-group pieces), PE-transposed
     in f32, cast to fp16 on the ACT PSUM->SBUF copy -> xt_sb [4096pos, 256ch].
     Patch table in DRAM: x_patch[lin] = [x_t[lin], x_t[lin+1], x_t[lin+64],
     x_t[lin+65]] (2KB rows) via 7 shifted strided DMA writes.
  2. Coords on DVE (f32): y0=floor(sy) (magic-number round + is_gt fix),
     base row r=clip(y0,0,62), col b=clip(x0,0,62), separable slot weights
     ws[4] reproducing mmcv zero-padding bilinear exactly (fp16 copies).
  3. Gather: one prepare_only dma_gather + trigger per (1024-pos chunk, tap):
     2KB elems from x_patch, alternating SWDGE queues; gpsimd only runs
     descriptor-gen, the drain is paced by the SDMA engines.
  4. Blend: corner products split ACT (corners 0,3: per-partition fp16 scale)
     and DVE (corners 1,2: broadcast tensor_tensor); 3 adds on DVE.
  5. PE-transpose blended [pos,ch]->[ch,pos] (fp16), PSUM->SBUF copies split
     ACT/DVE, then per-tap GEMM accumulation into 4 persistent PSUM banks
     (18 contraction blocks of 128, fp16 operands, f32 PSUM).
"""

import dataclasses

import numpy as np

_CACHE = {}

H = 64
W = 64
HW = 4096
C = 256
O = 256
K = 9
NCORES = 8
MAGIC = float(3 << 22)  # 1.5*2^23: keeps x+MAGIC in [2^23, 2^24) for |x|<2^22
USE_PREP_GATHER = False


def _step0(ap, inner):
    """Expand a [128, n] AP to [128, n, inner] with stride-0 inner dim."""
    return dataclasses.replace(ap, ap=list(ap.ap) + [[0, inner]])


def _emit(tc, nc, aps, rec=None, queue_plan=None):
    import contextlib

    import concourse.bass as bass
    import concourse.mybir as mybir
    from concourse.masks import make_identity

    dt = mybir.dt
    Alu = mybir.AluOpType
    Act = mybir.ActivationFunctionType

    x_in = aps["x"]          # [256, 4096] f32
    off_in = aps["offset"]   # [18, 4096]  f32
    w2_in = aps["w2"]        # [2304, 256] f32   (k-major, then c; lhsT layout)
    out_d = aps["out"]       # [256, 4096] f32

    ctx = contextlib.ExitStack()
    with ctx:
        # ---------------- pools ----------------
        cpool = ctx.enter_context(tc.tile_pool(name="cpool", bufs=1))
        dpool = ctx.enter_context(tc.tile_pool(name="dpool", bufs=1, space="DRAM"))

        # ---------------- persistent tiles ----------------
        ident16 = cpool.tile([128, 128], dt.float16, name="ident16")
        ident32 = cpool.tile([128, 128], dt.float32, name="ident32")
        make_identity(nc, ident16)
        make_identity(nc, ident32)

        w2_sb = cpool.tile([128, 18, 256], dt.float16, name="w2_sb")
        _i = nc.gpsimd.dma_start(
            out=w2_sb, in_=w2_in.rearrange("(kb ci) o -> ci kb o", ci=128)
        )
        if rec is not None:
            rec["plain"].append(_i.ins if hasattr(_i, "ins") else _i)
        # slot-weight fields [128 (p%128), st, K*32] and wrapped gather idx
        # ws16 holds corners 1,2 (DVE broadcast mults); ws32 holds corners
        # 0,3 in fp32 (ACT scale APs must be fp32)
        ws16 = cpool.tile([128, 4, K * 32], dt.float16, name="ws16")
        ws32 = cpool.tile([128, 2, K * 32], dt.float32, name="ws32")
        idxw = cpool.tile([128, K * 4 * 64], dt.int16, name="idxw")

        x_patch = dpool.tile([HW, 1024], dt.float16, name="x_patch")

        gsem = [nc.alloc_semaphore("gsem0"), nc.alloc_semaphore("gsem1")]

        # ================= PREP PHASE (scoped pools) =================
        with tc.tile_pool(name="prep", bufs=1) as pp, tc.tile_pool(
            name="ppsum", bufs=2, space="PSUM"
        ) as pps:
            eng = [nc.sync, nc.scalar]
            # ---- x load (f32, HWDGE, 4 pieces) ----
            x_sb = pp.tile([128, 2, HW], dt.float32, name="x_sb")
            xr = x_in.rearrange("(h c) p -> c h p", h=2)
            for piece in range(4):
                sl = slice(piece * 1024, (piece + 1) * 1024)
                eng[piece % 2].dma_start(out=x_sb[:, :, sl], in_=xr[:, :, sl])
            # ---- offsets load ----
            off_sb = pp.tile([18, HW], dt.float32, name="off_sb")
            nc.sync.dma_start(out=off_sb, in_=off_in)

            # ---- x transpose (f32) + cast-to-fp16 copies ----
            xt_sb = pp.tile([128, 32, C], dt.float16, name="xt_sb")
            for i in range(32):
                xtp = pps.tile([128, 256], dt.float32, name="xtp", tag="xtp")
                for h in range(2):
                    nc.tensor.transpose(
                        xtp[:, h * 128 : (h + 1) * 128],
                        x_sb[:, h, i * 128 : (i + 1) * 128],
                        ident32,
                    )
                nc.scalar.activation(xt_sb[:, i, :], xtp, Act.Copy)
            # ---- patch table: x_patch[lin, (s,t)*256:+256] = xt[lin+64s+t] ----
            # writes split between the two HWDGE engines (sync + scalar)
            for s in range(2):
                for t in range(2):
                    sh = 64 * s + t
                    slot = (2 * s + t) * 256
                    # rows p = i*128+j ; dst row p-sh for p >= sh
                    for half in range(2):
                        i0 = half * 16
                        dst_a = bass.AP(
                            tensor=x_patch.tensor,
                            offset=x_patch.offset + slot + i0 * 128 * 1024,
                            ap=[[1024, 128 - sh], [128 * 1024, 16], [1, 256]],
                        )
                        eng[(2 * s + t + half) % 2].dma_start(
                            out=dst_a, in_=xt_sb[sh:128, i0 : i0 + 16, :]
                        )
                    if sh:
                        dst_b = bass.AP(
                            tensor=x_patch.tensor,
                            offset=x_patch.offset + slot + (128 - sh) * 1024,
                            ap=[[1024, sh], [128 * 1024, 31], [1, 256]],
                        )
                        eng[(s + t) % 2].dma_start(
                            out=dst_b, in_=xt_sb[0:sh, 1:32, :]
                        )

            # ---- offsets -> p-major layout via PE transpose ----
            offp = pp.tile([128, 32, 18], dt.float32, name="offp")
            for i in range(32):
                pso = pps.tile([128, 18], dt.float32, name="pso", tag="pso")
                nc.tensor.transpose(
                    pso, off_sb[:, i * 128 : (i + 1) * 128], ident32[0:18, 0:18]
                )
                nc.vector.tensor_copy(offp[:, i, :], pso)

            # ---- position iota ----
            pos_i = pp.tile([128, 32], dt.int32, name="pos_i")
            nc.gpsimd.iota(pos_i, pattern=[[128, 32]], base=0, channel_multiplier=1)
            POS = pp.tile([128, 32], dt.float32, name="POS")
            nc.vector.tensor_copy(POS, pos_i)
            Pq = pp.tile([128, 32], dt.float32, name="Pq")
            nc.vector.tensor_scalar(Pq, POS, 1.0 / 64.0, None, Alu.mult)
            I_ = pp.tile([128, 32], dt.float32, name="I_")
            CMP = pp.tile([128, 32], dt.float32, name="CMPij")
            nc.vector.tensor_scalar(CMP, Pq, MAGIC, None, Alu.add)
            nc.vector.tensor_scalar(I_, CMP, MAGIC, None, Alu.subtract)
            nc.vector.tensor_tensor(CMP, I_, Pq, Alu.is_gt)
            nc.vector.tensor_tensor(I_, I_, CMP, Alu.subtract)
            J_ = pp.tile([128, 32], dt.float32, name="J_")
            nc.vector.scalar_tensor_tensor(J_, I_, -64.0, POS, Alu.mult, Alu.add)

            # ---- per-axis coordinate pipeline ----
            KI = [k // 3 for k in range(K)]
            KJ = [k % 3 for k in range(K)]

            def axis_pipeline(off_field, base_tile, kshift, L, WS0, WS1, R_out):
                F = K * 32
                S = pp.tile([128, F], dt.float32, name=f"S{L}", tag=f"S{L}")
                for k in range(K):
                    nc.vector.scalar_tensor_tensor(
                        S[:, k * 32 : (k + 1) * 32],
                        off_field(k),
                        float(kshift[k] - 1),
                        base_tile,
                        Alu.add,
                        Alu.add,
                    )
                t = lambda nm: pp.tile([128, F], dt.float32, name=nm, tag=nm)
                Y0 = t(f"Y0{L}")
                Ct = t(f"Ct{L}")
                nc.vector.tensor_scalar(Ct, S, MAGIC, None, Alu.add)
                nc.vector.tensor_scalar(Y0, Ct, MAGIC, None, Alu.subtract)
                nc.vector.tensor_tensor(Ct, Y0, S, Alu.is_gt)
                nc.vector.tensor_tensor(Y0, Y0, Ct, Alu.subtract)
                LY = t(f"LY{L}")
                nc.vector.tensor_tensor(LY, S, Y0, Alu.subtract)
                WY0 = t(f"WY0{L}")
                nc.vector.tensor_scalar(WY0, LY, -1.0, 1.0, Alu.mult, Alu.add)
                V0 = t(f"V0{L}")
                V1 = t(f"V1{L}")
                nc.vector.tensor_scalar(V0, Y0, 0.0, None, Alu.is_ge)
                nc.vector.tensor_scalar(Ct, Y0, 63.0, None, Alu.is_le)
                nc.vector.tensor_tensor(V0, V0, Ct, Alu.mult)
                nc.vector.tensor_scalar(V1, Y0, -1.0, None, Alu.is_ge)
                nc.vector.tensor_scalar(Ct, Y0, 62.0, None, Alu.is_le)
                nc.vector.tensor_tensor(V1, V1, Ct, Alu.mult)
                nc.vector.tensor_tensor(WY0, WY0, V0, Alu.mult)
                nc.vector.tensor_tensor(LY, LY, V1, Alu.mult)
                R = R_out
                nc.vector.tensor_scalar(R, Y0, 0.0, 62.0, Alu.max, Alu.min)
                C0 = t(f"C0{L}")
                C1 = t(f"C1{L}")
                nc.vector.tensor_scalar(C0, Y0, 0.0, 63.0, Alu.max, Alu.min)
                nc.vector.tensor_scalar(C1, Y0, 1.0, 0.0, Alu.add, Alu.max)
                nc.vector.tensor_scalar(C1, C1, 63.0, None, Alu.min)
                E = t(f"E{L}")
                T1 = t(f"T1{L}")
                nc.vector.tensor_tensor(E, C0, R, Alu.is_equal)
                nc.vector.tensor_tensor(T1, WY0, E, Alu.mult)
                nc.vector.tensor_tensor(E, C1, R, Alu.is_equal)
                nc.vector.tensor_tensor(E, LY, E, Alu.mult)
                nc.vector.tensor_tensor(WS0, T1, E, Alu.add)
                Rp = t(f"Rp{L}")
                nc.vector.tensor_scalar(Rp, R, 1.0, None, Alu.add)
                nc.vector.tensor_tensor(E, C0, Rp, Alu.is_equal)
                nc.vector.tensor_tensor(T1, WY0, E, Alu.mult)
                nc.vector.tensor_tensor(E, C1, Rp, Alu.is_equal)
                nc.vector.tensor_tensor(E, LY, E, Alu.mult)
                nc.vector.tensor_tensor(WS1, T1, E, Alu.add)

            F = K * 32
            WSY0 = pp.tile([128, F], dt.float32, name="WSY0")
            WSY1 = pp.tile([128, F], dt.float32, name="WSY1")
            WSX0 = pp.tile([128, F], dt.float32, name="WSX0")
            WSX1 = pp.tile([128, F], dt.float32, name="WSX1")
            RY = pp.tile([128, F], dt.float32, name="RY")
            RX = pp.tile([128, F], dt.float32, name="RX")
            axis_pipeline(lambda k: offp[:, :, 2 * k], I_, KI, "y", WSY0, WSY1, RY)
            axis_pipeline(
                lambda k: offp[:, :, 2 * k + 1], J_, KJ, "x", WSX0, WSX1, RX
            )
            WSf = pp.tile([128, F], dt.float32, name="WSf", tag="WSf")
            nc.vector.tensor_tensor(ws32[:, 0, :], WSY0, WSX0, Alu.mult)
            nc.vector.tensor_tensor(ws32[:, 1, :], WSY1, WSX1, Alu.mult)
            nc.vector.tensor_copy(ws16[:, 3, :], ws32[:, 1, :])
            for st, (wy, wx) in [(1, (WSY0, WSX1)), (2, (WSY1, WSX0))]:
                nc.vector.tensor_tensor(WSf, wy, wx, Alu.mult)
                nc.vector.tensor_copy(ws16[:, st, :], WSf)

            # ---- gather indices: lin = RY*64 + RX, cast to i16 ----
            IDX = pp.tile([128, 384], dt.float32, name="IDX")
            nc.gpsimd.memset(IDX, 0)
            nc.vector.scalar_tensor_tensor(
                IDX[:, 0:F], RY, 64.0, RX, Alu.mult, Alu.add
            )
            # shuffle p%128 -> p%16 wrap via two PE transpose stages (f32),
            # casting to i16 on the final PSUM->SBUF copy:
            # idxw[t, (k,ch)*64 + bl*8 + g] = IDX[g*16+t, k*32+ch*8+bl]
            t1sb = pp.tile([128, 3, 128], dt.float32, name="t1sb")
            for ct in range(3):
                ps1 = pps.tile([128, 128], dt.float32, name="ps1", tag="ps1")
                nc.tensor.transpose(ps1, IDX[:, ct * 128 : (ct + 1) * 128], ident32)
                nc.vector.tensor_copy(t1sb[:, ct, :], ps1)
            # stage 2: per (ct, g): [128col, 16] -> [16, 128col]
            for ct in range(3):
                nk = 4 if ct < 2 else 1  # k-count covered by this col tile
                for g in range(8):
                    ps2 = pps.tile([16, 128], dt.float32, name="ps2", tag="ps2")
                    nc.tensor.transpose(
                        ps2, t1sb[:, ct, g * 16 : (g + 1) * 16], ident32
                    )
                    # dst cols: for k' in [0,nk), ch in 4, bl in 8:
                    #   ((ct*4+k')*4+ch)*64 + bl*8 + g
                    dst = bass.AP(
                        tensor=idxw.tensor,
                        offset=idxw.offset + (ct * 4 * 4) * 64 + g,
                        ap=[[idxw.ap[0][0], 16], [256, nk], [64, 4], [8, 8]],
                    )
                    nc.vector.tensor_copy(
                        dst,
                        ps2[0:16, 0 : nk * 32].rearrange(
                            "t (k c b) -> t k c b", k=nk, c=4
                        ),
                    )
            for rep in range(1, 8):
                eng[rep % 2].dma_start(
                    out=idxw[rep * 16 : (rep + 1) * 16, :], in_=idxw[0:16, :]
                )

        # ================= MAIN LOOP =================
        pgpool = ctx.enter_context(tc.tile_pool(name="pgpool", bufs=1, space="PSUM"))
        ptpool = ctx.enter_context(tc.tile_pool(name="ptpool", bufs=3, space="PSUM"))
        gpool = ctx.enter_context(tc.tile_pool(name="gpool", bufs=4))
        spool = ctx.enter_context(tc.tile_pool(name="spool", bufs=3))
        bpool = ctx.enter_context(tc.tile_pool(name="bpool", bufs=3))
        opool = ctx.enter_context(tc.tile_pool(name="opool", bufs=2))

        for ch in range(4):  # 1024-position chunks
            pg = [
                pgpool.tile([128, 512], dt.float32, name=f"pg{ms}", tag=f"pg{ms}")
                for ms in range(4)
            ]
            for k in range(K):
                G = gpool.tile([128, 8, 1024], dt.float16, name="G", tag="G")
                qi = ch * K + k
                q = 0 if queue_plan is None else queue_plan[qi]
                if USE_PREP_GATHER:
                    _i = nc.gpsimd.dma_gather(
                        G,
                        x_patch,
                        idxw[:, (k * 4 + ch) * 64 : (k * 4 + ch + 1) * 64],
                        num_idxs=1024,
                        num_idxs_reg=1024,
                        elem_size=1024,
                        elem_step=1024,
                        queue_num=q,
                        prepare_only=True,
                        sem=gsem[q],
                    )
                    nc.gpsimd.trigger_dma(count=1, queue_num=q)
                else:
                    _i = nc.gpsimd.dma_gather(
                        G,
                        x_patch,
                        idxw[:, (k * 4 + ch) * 64 : (k * 4 + ch + 1) * 64],
                        num_idxs=1024,
                        num_idxs_reg=1024,
                        elem_size=1024,
                        elem_step=1024,
                        queue_num=q,
                    )
                if rec is not None:
                    rec["gather"].append(_i.ins if hasattr(_i, "ins") else _i)
                # blend 4 corners: A = sum_st ws_st * G[:, :, st].
                # corners 0,3 on ACT (per-partition fp16 scale, per-bl ops);
                # corners 1,2 on DVE as fused broadcast-mults (step-0 in1).
                # A accumulates corners 0..2 + low half of corner 3; the high
                # half of corner 3 lands in P3 and is summed into the PSUM
                # transpose (accumulating matmul) instead of a DVE add.
                A = bpool.tile([128, 8, 256], dt.float16, name="A", tag="A")
                Mt = bpool.tile([128, 8, 256], dt.float16, name="Mt", tag="Mt")
                P0 = bpool.tile([128, 8, 256], dt.float16, name="P0", tag="P0")
                P3 = bpool.tile([128, 8, 256], dt.float16, name="P3", tag="P3")
                for bl in range(8):
                    wc = k * 32 + ch * 8 + bl
                    nc.scalar.activation(
                        P0[:, bl, :],
                        G[:, bl, 0:256],
                        Act.Copy,
                        scale=ws32[:, 0, wc : wc + 1],
                    )
                    nc.scalar.activation(
                        P3[:, bl, :],
                        G[:, bl, 768:1024],
                        Act.Copy,
                        scale=ws32[:, 1, wc : wc + 1],
                    )
                wsl = lambda st: ws16[:, st, k * 32 + ch * 8 : k * 32 + (ch + 1) * 8]
                nc.vector.tensor_tensor(
                    A, G[:, :, 256:512], _step0(wsl(1), 256), Alu.mult
                )
                nc.vector.tensor_tensor(A, A, P0, Alu.add)
                nc.vector.tensor_tensor(
                    Mt, G[:, :, 512:768], _step0(wsl(2), 256), Alu.mult
                )
                nc.vector.tensor_tensor(A, A, Mt, Alu.add)
                nc.vector.tensor_tensor(A, A, P3, Alu.add)
                # transpose [pos, ch] -> [ch, pos], accumulating A + P3 in
                # PSUM; PSUM->SBUF copies split ACT (h=0) / DVE (h=1); then
                # per-tap GEMM accumulation.
                Ssb = spool.tile([128, 2, 1024], dt.float16, name="Ssb", tag="Ssb")
                for h in range(2):
                    for blq in range(2):
                        pt = ptpool.tile(
                            [128, 512], dt.float16, name="pt", tag="pt"
                        )
                        for bb in range(4):
                            bl = blq * 4 + bb
                            nc.tensor.transpose(
                                pt[:, bb * 128 : (bb + 1) * 128],
                                A[:, bl, h * 128 : (h + 1) * 128],
                                ident16,
                            )
                        dst = Ssb[:, h, blq * 512 : (blq + 1) * 512]
                        if h == 0:
                            nc.scalar.activation(dst, pt, Act.Copy)
                        else:
                            nc.vector.tensor_copy(dst, pt)
                for h in range(2):
                    kb = 2 * k + h
                    for m in range(2):
                        for sub in range(2):
                            nc.tensor.matmul(
                                pg[2 * m + sub],
                                lhsT=w2_sb[:, kb, m * 128 : (m + 1) * 128],
                                rhs=Ssb[:, h, sub * 512 : (sub + 1) * 512],
                                start=(kb == 0),
                                stop=(kb == 17),
                            )
            # PSUM evict + output store for this chunk
            for m in range(2):
                for sub in range(2):
                    ot = opool.tile([128, 512], dt.float32, name="ot", tag="ot")
                    nc.vector.tensor_copy(ot, pg[2 * m + sub])
                    nc.sync.dma_start(
                        out=out_d[
                            m * 128 : (m + 1) * 128,
                            ch * 1024 + sub * 512 : ch * 1024 + (sub + 1) * 512,
                        ],
                        in_=ot,
                    )


def _lane_of(inst):
    from concourse.tile_sem_assignment import PROC_NAME_TO_IDX

    rev = {v: k for k, v in PROC_NAME_TO_IDX.items()}
    nm = rev.get(inst.bass_scheduled_proc, "")
    return int(nm[5:]) if nm.startswith("DMASW") else None


def build(queue_plan="auto"):
    import concourse.mybir as mybir
    from concourse import bacc, tile

    dt = mybir.dt
    nc = bacc.Bacc(
        "TRN2",
        target_bir_lowering=False,
        debug=False,
        enable_asserts=False,
        num_devices=NCORES,
        num_swdge_queues=2,
    )
    aps = {
        "x": nc.dram_tensor("x", [C, HW], dt.float32, kind="ExternalInput").ap(),
        "offset": nc.dram_tensor(
            "offset", [2 * K, HW], dt.float32, kind="ExternalInput"
        ).ap(),
        "w2": nc.dram_tensor(
            "w2", [C * K, O], dt.float32, kind="ExternalInput"
        ).ap(),
        "out": nc.dram_tensor(
            "out", [O, HW], dt.float32, kind="ExternalOutput"
        ).ap(),
    }
    if queue_plan == "auto":
        # pass 1: discover each SWDGE DMA's DMASW lane, then rebuild with a
        # lane-consistent queue assignment (lane%2, forced 0 on lanes that
        # host plain queue-0 dma_starts).
        rec = {"gather": [], "plain": []}
        with tile.TileContext(nc) as tc:
            _emit(tc, nc, aps, rec=rec, queue_plan=None)
        plain_lanes = {_lane_of(i) for i in rec["plain"]}
        plan = []
        for gi in rec["gather"]:
            lane = _lane_of(gi)
            q = 0 if (lane is None or lane in plain_lanes) else lane % 2
            plan.append(q)
        return build(plan)
    with tile.TileContext(nc) as tc:
        _emit(tc, nc, aps, queue_plan=queue_plan)
    nc.compile()
    return nc


def prep_in_maps(x, offset, weight):
    x = np.asarray(x, dtype=np.float32)
    offset = np.asarray(offset, dtype=np.float32)
    weight = np.asarray(weight, dtype=np.float32)
    w2 = np.ascontiguousarray(
        weight.reshape(O, C, K).transpose(2, 1, 0).reshape(C * K, O)
    )
    in_maps = []
    for b in range(NCORES):
        in_maps.append(
            {
                "x": np.ascontiguousarray(x[b].reshape(C, HW)),
                "offset": np.ascontiguousarray(offset[b].reshape(2 * K, HW)),
                "w2": w2,
            }
        )
    return in_maps


def run(x, offset, weight, trace=False, **kw):
    from concourse import bass_utils

    if "nc" not in _CACHE:
        _CACHE["nc"] = build()
    nc = _CACHE["nc"]
    res = bass_utils.run_bass_kernel_spmd(
        nc, prep_in_maps(x, offset, weight), core_ids=list(range(NCORES)),
        trace=trace, **kw,
    )
    out = np.stack([r["out"].reshape(O, H, W) for r in res.results])
    return out, res


def kernel(x, offset, weight):
    out, _ = run(x, offset, weight, trace=False)
    return out


# revision 30
# speedup vs baseline: 1.2721x; 1.0441x over previous
"""DeformConv (B=8, C=256, H=W=64, O=256, 3x3, DG=1) Trainium2 Bass kernel.

Sharding: data-parallel over batch, one batch element per NeuronCore (8 cores).

Per-core pipeline (B=1):
  1. x [256,4096] f32 loaded via HWDGE (4 column-group pieces), PE-transposed
     in f32, cast to fp16 on the ACT PSUM->SBUF copy -> xt_sb [4096pos, 256ch].
     Patch table in DRAM: x_patch[lin] = [x_t[lin], x_t[lin+1], x_t[lin+64],
     x_t[lin+65]] (2KB rows) via 7 shifted strided DMA writes.
  2. Coords on DVE (f32): y0=floor(sy) (magic-number round + is_gt fix),
     base row r=clip(y0,0,62), col b=clip(x0,0,62), separable slot weights
     ws[4] reproducing mmcv zero-padding bilinear exactly (fp16 copies).
  3. Gather: one prepare_only dma_gather + trigger per (1024-pos chunk, tap):
     2KB elems from x_patch, alternating SWDGE queues; gpsimd only runs
     descriptor-gen, the drain is paced by the SDMA engines.
  4. Blend: corner products split ACT (corners 0,3: per-partition fp16 scale)
     and DVE (corners 1,2: broadcast tensor_tensor); 3 adds on DVE.
  5. PE-transpose blended [pos,ch]->[ch,pos] (fp16), PSUM->SBUF copies split
     ACT/DVE, then per-tap GEMM accumulation into 4 persistent PSUM banks
     (18 contraction blocks of 128, fp16 operands, f32 PSUM).
"""

import dataclasses

import numpy as np

_CACHE = {}

H = 64
W = 64
HW = 4096
C = 256
O = 256
K = 9
NCORES = 8
MAGIC = float(3 << 22)  # 1.5*2^23: keeps x+MAGIC in [2^23, 2^24) for |x|<2^22
USE_PREP_GATHER = False


def _step0(ap, inner):
    """Expand a [128, n] AP to [128, n, inner] with stride-0 inner dim."""
    return dataclasses.replace(ap, ap=list(ap.ap) + [[0, inner]])


def _emit(tc, nc, aps, rec=None, queue_plan=None):
    import contextlib

    import concourse.bass as bass
    import concourse.mybir as mybir
    from concourse.masks import make_identity

    dt = mybir.dt
    Alu = mybir.AluOpType
    Act = mybir.ActivationFunctionType

    x_in = aps["x"]          # [256, 4096] f32
    off_in = aps["offset"]   # [18, 4096]  f32
    w2_in = aps["w2"]        # [2304, 256] f32   (k-major, then c; lhsT layout)
    out_d = aps["out"]       # [256, 4096] f32

    ctx = contextlib.ExitStack()
    with ctx:
        # ---------------- pools ----------------
        cpool = ctx.enter_context(tc.tile_pool(name="cpool", bufs=1))
        dpool = ctx.enter_context(tc.tile_pool(name="dpool", bufs=1, space="DRAM"))

        # ---------------- persistent tiles ----------------
        ident16 = cpool.tile([128, 128], dt.float16, name="ident16")
        ident32 = cpool.tile([128, 128], dt.float32, name="ident32")
        make_identity(nc, ident16)
        make_identity(nc, ident32)

        w2_sb = cpool.tile([128, 18, 256], dt.float16, name="w2_sb")
        _i = nc.gpsimd.dma_start(
            out=w2_sb, in_=w2_in.rearrange("(kb ci) o -> ci kb o", ci=128)
        )
        if rec is not None:
            rec["plain"].append(_i.ins if hasattr(_i, "ins") else _i)
        # slot-weight fields [128 (p%128), st, K*32] and wrapped gather idx
        # ws16 holds corners 1,2 (DVE broadcast mults); ws32 holds corners
        # 0,3 in fp32 (ACT scale APs must be fp32)
        ws16 = cpool.tile([128, 4, K * 32], dt.float16, name="ws16")
        ws32 = cpool.tile([128, 2, K * 32], dt.float32, name="ws32")
        idxw = cpool.tile([128, K * 4 * 64], dt.int16, name="idxw")

        x_patch = dpool.tile([HW, 1024], dt.float16, name="x_patch")

        gsem = [nc.alloc_semaphore("gsem0"), nc.alloc_semaphore("gsem1")]

        # ================= PREP PHASE (scoped pools) =================
        with tc.tile_pool(name="prep", bufs=1) as pp, tc.tile_pool(
            name="ppsum", bufs=2, space="PSUM"
        ) as pps:
            eng = [nc.sync, nc.scalar]
            # ---- x load (f32, HWDGE, 4 pieces) ----
            x_sb = pp.tile([128, 2, HW], dt.float32, name="x_sb")
            xr = x_in.rearrange("(h c) p -> c h p", h=2)
            for piece in range(4):
                sl = slice(piece * 1024, (piece + 1) * 1024)
                eng[piece % 2].dma_start(out=x_sb[:, :, sl], in_=xr[:, :, sl])
            # ---- offsets load ----
            off_sb = pp.tile([18, HW], dt.float32, name="off_sb")
            nc.sync.dma_start(out=off_sb, in_=off_in)

            # ---- x transpose (f32) + cast-to-fp16 copies ----
            xt_sb = pp.tile([128, 32, C], dt.float16, name="xt_sb")
            for i in range(32):
                xtp = pps.tile([128, 256], dt.float32, name="xtp", tag="xtp")
                for h in range(2):
                    nc.tensor.transpose(
                        xtp[:, h * 128 : (h + 1) * 128],
                        x_sb[:, h, i * 128 : (i + 1) * 128],
                        ident32,
                    )
                nc.scalar.activation(xt_sb[:, i, :], xtp, Act.Copy)
            # ---- patch table: x_patch[lin, (s,t)*256:+256] = xt[lin+64s+t] ----
            # writes split between the two HWDGE engines (sync + scalar)
            for s in range(2):
                for t in range(2):
                    sh = 64 * s + t
                    slot = (2 * s + t) * 256
                    # rows p = i*128+j ; dst row p-sh for p >= sh
                    for half in range(2):
                        i0 = half * 16
                        dst_a = bass.AP(
                            tensor=x_patch.tensor,
                            offset=x_patch.offset + slot + i0 * 128 * 1024,
                            ap=[[1024, 128 - sh], [128 * 1024, 16], [1, 256]],
                        )
                        eng[(2 * s + t + half) % 2].dma_start(
                            out=dst_a, in_=xt_sb[sh:128, i0 : i0 + 16, :]
                        )
                    if sh:
                        dst_b = bass.AP(
                            tensor=x_patch.tensor,
                            offset=x_patch.offset + slot + (128 - sh) * 1024,
                            ap=[[1024, sh], [128 * 1024, 31], [1, 256]],
                        )
                        eng[(s + t) % 2].dma_start(
                            out=dst_b, in_=xt_sb[0:sh, 1:32, :]
                        )

            # ---- offsets -> p-major layout via PE transpose ----
            offp = pp.tile([128, 32, 18], dt.float32, name="offp")
            for i in range(32):
                pso = pps.tile([128, 18], dt.float32, name="pso", tag="pso")
                nc.tensor.transpose(
                    pso, off_sb[:, i * 128 : (i + 1) * 128], ident32[0:18, 0:18]
                )
                nc.vector.tensor_copy(offp[:, i, :], pso)

            # ---- position iota ----
            pos_i = pp.tile([128, 32], dt.int32, name="pos_i")
            nc.gpsimd.iota(pos_i, pattern=[[128, 32]], base=0, channel_multiplier=1)
            POS = pp.tile([128, 32], dt.float32, name="POS")
            nc.vector.tensor_copy(POS, pos_i)
            Pq = pp.tile([128, 32], dt.float32, name="Pq")
            nc.vector.tensor_scalar(Pq, POS, 1.0 / 64.0, None, Alu.mult)
            I_ = pp.tile([128, 32], dt.float32, name="I_")
            CMP = pp.tile([128, 32], dt.float32, name="CMPij")
            nc.vector.tensor_scalar(CMP, Pq, MAGIC, None, Alu.add)
            nc.vector.tensor_scalar(I_, CMP, MAGIC, None, Alu.subtract)
            nc.vector.tensor_tensor(CMP, I_, Pq, Alu.is_gt)
            nc.vector.tensor_tensor(I_, I_, CMP, Alu.subtract)
            J_ = pp.tile([128, 32], dt.float32, name="J_")
            nc.vector.scalar_tensor_tensor(J_, I_, -64.0, POS, Alu.mult, Alu.add)

            # ---- per-axis coordinate pipeline ----
            KI = [k // 3 for k in range(K)]
            KJ = [k % 3 for k in range(K)]

            def axis_pipeline(off_field, base_tile, kshift, L, WS0, WS1, R_out):
                F = K * 32
                S = pp.tile([128, F], dt.float32, name=f"S{L}", tag=f"S{L}")
                for k in range(K):
                    nc.vector.scalar_tensor_tensor(
                        S[:, k * 32 : (k + 1) * 32],
                        off_field(k),
                        float(kshift[k] - 1),
                        base_tile,
                        Alu.add,
                        Alu.add,
                    )
                t = lambda nm: pp.tile([128, F], dt.float32, name=nm, tag=nm)
                Y0 = t(f"Y0{L}")
                Ct = t(f"Ct{L}")
                nc.vector.tensor_scalar(Ct, S, MAGIC, None, Alu.add)
                nc.vector.tensor_scalar(Y0, Ct, MAGIC, None, Alu.subtract)
                nc.vector.tensor_tensor(Ct, Y0, S, Alu.is_gt)
                nc.vector.tensor_tensor(Y0, Y0, Ct, Alu.subtract)
                LY = t(f"LY{L}")
                nc.vector.tensor_tensor(LY, S, Y0, Alu.subtract)
                WY0 = t(f"WY0{L}")
                nc.vector.tensor_scalar(WY0, LY, -1.0, 1.0, Alu.mult, Alu.add)
                V0 = t(f"V0{L}")
                V1 = t(f"V1{L}")
                nc.vector.tensor_scalar(V0, Y0, 0.0, None, Alu.is_ge)
                nc.vector.tensor_scalar(Ct, Y0, 63.0, None, Alu.is_le)
                nc.vector.tensor_tensor(V0, V0, Ct, Alu.mult)
                nc.vector.tensor_scalar(V1, Y0, -1.0, None, Alu.is_ge)
                nc.vector.tensor_scalar(Ct, Y0, 62.0, None, Alu.is_le)
                nc.vector.tensor_tensor(V1, V1, Ct, Alu.mult)
                nc.vector.tensor_tensor(WY0, WY0, V0, Alu.mult)
                nc.vector.tensor_tensor(LY, LY, V1, Alu.mult)
                R = R_out
                nc.vector.tensor_scalar(R, Y0, 0.0, 62.0, Alu.max, Alu.min)
                C0 = t(f"C0{L}")
                C1 = t(f"C1{L}")
                nc.vector.tensor_scalar(C0, Y0, 0.0, 63.0, Alu.max, Alu.min)
                nc.vector.tensor_scalar(C1, Y0, 1.0, 0.0, Alu.add, Alu.max)
                nc.vector.tensor_scalar(C1, C1, 63.0, None, Alu.min)
                E = t(f"E{L}")
                T1 = t(f"T1{L}")
                nc.vector.tensor_tensor(E, C0, R, Alu.is_equal)
                nc.vector.tensor_tensor(T1, WY0, E, Alu.mult)
                nc.vector.tensor_tensor(E, C1, R, Alu.is_equal)
                nc.vector.tensor_tensor(E, LY, E, Alu.mult)
                nc.vector.tensor_tensor(WS0, T1, E, Alu.add)
                Rp = t(f"Rp{L}")
                nc.vector.tensor_scalar(Rp, R, 1.0, None, Alu.add)
                nc.vector.tensor_tensor(E, C0, Rp, Alu.is_equal)
                nc.vector.tensor_tensor(T1, WY0, E, Alu.mult)
                nc.vector.tensor_tensor(E, C1, Rp, Alu.is_equal)
                nc.vector.tensor_tensor(E, LY, E, Alu.mult)
                nc.vector.tensor_tensor(WS1, T1, E, Alu.add)

            F = K * 32
            WSY0 = pp.tile([128, F], dt.float32, name="WSY0")
            WSY1 = pp.tile([128, F], dt.float32, name="WSY1")
            WSX0 = pp.tile([128, F], dt.float32, name="WSX0")
            WSX1 = pp.tile([128, F], dt.float32, name="WSX1")
            RY = pp.tile([128, F], dt.float32, name="RY")
            RX = pp.tile([128, F], dt.float32, name="RX")
            axis_pipeline(lambda k: offp[:, :, 2 * k], I_, KI, "y", WSY0, WSY1, RY)
            axis_pipeline(
                lambda k: offp[:, :, 2 * k + 1], J_, KJ, "x", WSX0, WSX1, RX
            )
            WSf = pp.tile([128, F], dt.float32, name="WSf", tag="WSf")
            nc.vector.tensor_tensor(ws32[:, 0, :], WSY0, WSX0, Alu.mult)
            nc.vector.tensor_tensor(ws32[:, 1, :], WSY1, WSX1, Alu.mult)
            nc.vector.tensor_copy(ws16[:, 3, :], ws32[:, 1, :])
            for st, (wy, wx) in [(1, (WSY0, WSX1)), (2, (WSY1, WSX0))]:
                nc.vector.tensor_tensor(WSf, wy, wx, Alu.mult)
                nc.vector.tensor_copy(ws16[:, st, :], WSf)

            # ---- gather indices: lin = RY*64 + RX, cast to i16 ----
            IDX = pp.tile([128, 384], dt.float32, name="IDX")
            nc.gpsimd.memset(IDX, 0)
            nc.vector.scalar_tensor_tensor(
                IDX[:, 0:F], RY, 64.0, RX, Alu.mult, Alu.add
            )
            # shuffle p%128 -> p%16 wrap via two PE transpose stages (f32),
            # casting to i16 on the final PSUM->SBUF copy:
            # idxw[t, (k,ch)*64 + bl*8 + g] = IDX[g*16+t, k*32+ch*8+bl]
            t1sb = pp.tile([128, 3, 128], dt.float32, name="t1sb")
            for ct in range(3):
                ps1 = pps.tile([128, 128], dt.float32, name="ps1", tag="ps1")
                nc.tensor.transpose(ps1, IDX[:, ct * 128 : (ct + 1) * 128], ident32)
                nc.vector.tensor_copy(t1sb[:, ct, :], ps1)
            # stage 2: per (ct, g): [128col, 16] -> [16, 128col]
            for ct in range(3):
                nk = 4 if ct < 2 else 1  # k-count covered by this col tile
                for g in range(8):
                    ps2 = pps.tile([16, 128], dt.float32, name="ps2", tag="ps2")
                    nc.tensor.transpose(
                        ps2, t1sb[:, ct, g * 16 : (g + 1) * 16], ident32
                    )
                    # dst cols: for k' in [0,nk), ch in 4, bl in 8:
                    #   ((ct*4+k')*4+ch)*64 + bl*8 + g
                    dst = bass.AP(
                        tensor=idxw.tensor,
                        offset=idxw.offset + (ct * 4 * 4) * 64 + g,
                        ap=[[idxw.ap[0][0], 16], [256, nk], [64, 4], [8, 8]],
                    )
                    nc.vector.tensor_copy(
                        dst,
                        ps2[0:16, 0 : nk * 32].rearrange(
                            "t (k c b) -> t k c b", k=nk, c=4
                        ),
                    )
            for rep in range(1, 8):
                eng[rep % 2].dma_start(
                    out=idxw[rep * 16 : (rep + 1) * 16, :], in_=idxw[0:16, :]
                )

        # ================= MAIN LOOP =================
        pgpool = ctx.enter_context(tc.tile_pool(name="pgpool", bufs=1, space="PSUM"))
        ptpool = ctx.enter_context(tc.tile_pool(name="ptpool", bufs=3, space="PSUM"))
        gpool = ctx.enter_context(tc.tile_pool(name="gpool", bufs=4))
        spool = ctx.enter_context(tc.tile_pool(name="spool", bufs=3))
        bpool = ctx.enter_context(tc.tile_pool(name="bpool", bufs=3))
        opool = ctx.enter_context(tc.tile_pool(name="opool", bufs=2))

        for ch in range(4):  # 1024-position chunks
            pg = [
                pgpool.tile([128, 512], dt.float32, name=f"pg{ms}", tag=f"pg{ms}")
                for ms in range(4)
            ]
            for k in range(K):
                G = gpool.tile([128, 8, 1024], dt.float16, name="G", tag="G")
                qi = ch * K + k
                q = 0 if queue_plan is None else queue_plan[qi]
                if USE_PREP_GATHER:
                    _i = nc.gpsimd.dma_gather(
                        G,
                        x_patch,
                        idxw[:, (k * 4 + ch) * 64 : (k * 4 + ch + 1) * 64],
                        num_idxs=1024,
                        num_idxs_reg=1024,
                        elem_size=1024,
                        elem_step=1024,
                        queue_num=q,
                        prepare_only=True,
                        sem=gsem[q],
                    )
                    nc.gpsimd.trigger_dma(count=1, queue_num=q)
                else:
                    _i = nc.gpsimd.dma_gather(
                        G,
                        x_patch,
                        idxw[:, (k * 4 + ch) * 64 : (k * 4 + ch + 1) * 64],
                        num_idxs=1024,
                        num_idxs_reg=1024,
                        elem_size=1024,
                        elem_step=1024,
                        queue_num=q,
                    )
                if rec is not None:
                    rec["gather"].append(_i.ins if hasattr(_i, "ins") else _i)
                # blend 4 corners: A = sum_st ws_st * G[:, :, st].
                # corners 0,3 on ACT (per-partition fp16 scale, per-bl ops);
                # corners 1,2 on DVE as fused broadcast-mults (step-0 in1).
                # A accumulates corners 0..2 + low half of corner 3; the high
                # half of corner 3 lands in P3 and is summed into the PSUM
                # transpose (accumulating matmul) instead of a DVE add.
                A = bpool.tile([128, 8, 256], dt.float16, name="A", tag="A")
                Mt = bpool.tile([128, 8, 256], dt.float16, name="Mt", tag="Mt")
                P0 = bpool.tile([128, 8, 256], dt.float16, name="P0", tag="P0")
                P3 = bpool.tile([128, 8, 256], dt.float16, name="P3", tag="P3")
                for bl in range(8):
                    wc = k * 32 + ch * 8 + bl
                    nc.scalar.activation(
                        P0[:, bl, :],
                        G[:, bl, 0:256],
                        Act.Copy,
                        scale=ws32[:, 0, wc : wc + 1],
                    )
                    nc.scalar.activation(
                        P3[:, bl, :],
                        G[:, bl, 768:1024],
                        Act.Copy,
                        scale=ws32[:, 1, wc : wc + 1],
                    )
                wsl = lambda st: ws16[:, st, k * 32 + ch * 8 : k * 32 + (ch + 1) * 8]
                nc.vector.tensor_tensor(
                    A, G[:, :, 256:512], _step0(wsl(1), 256), Alu.mult
                )
                nc.vector.tensor_tensor(A, A, P0, Alu.add)
                nc.vector.tensor_tensor(
                    Mt, G[:, :, 512:768], _step0(wsl(2), 256), Alu.mult
                )
                nc.vector.tensor_tensor(A, A, Mt, Alu.add)
                nc.vector.tensor_tensor(A, A, P3, Alu.add)
                # transpose [pos, ch] -> [ch, pos], accumulating A + P3 in
                # PSUM; PSUM->SBUF copies split ACT (h=0) / DVE (h=1); then
                # per-tap GEMM accumulation.
                Ssb = spool.tile([128, 2, 1024], dt.float16, name="Ssb", tag="Ssb")
                for h in range(2):
                    for blq in range(2):
                        pt = ptpool.tile(
                            [128, 512], dt.float16, name="pt", tag="pt"
                        )
                        for bb in range(4):
                            bl = blq * 4 + bb
                            nc.tensor.transpose(
                                pt[:, bb * 128 : (bb + 1) * 128],
                                A[:, bl, h * 128 : (h + 1) * 128],
                                ident16,
                            )
                        dst = Ssb[:, h, blq * 512 : (blq + 1) * 512]
                        nc.vector.tensor_copy(dst, pt)
                for h in range(2):
                    kb = 2 * k + h
                    for m in range(2):
                        for sub in range(2):
                            nc.tensor.matmul(
                                pg[2 * m + sub],
                                lhsT=w2_sb[:, kb, m * 128 : (m + 1) * 128],
                                rhs=Ssb[:, h, sub * 512 : (sub + 1) * 512],
                                start=(kb == 0),
                                stop=(kb == 17),
                            )
            # PSUM evict + output store for this chunk
            for m in range(2):
                for sub in range(2):
                    ot = opool.tile([128, 512], dt.float32, name="ot", tag="ot")
                    nc.vector.tensor_copy(ot, pg[2 * m + sub])
                    nc.sync.dma_start(
                        out=out_d[
                            m * 128 : (m + 1) * 128,
                            ch * 1024 + sub * 512 : ch * 1024 + (sub + 1) * 512,
                        ],
                        in_=ot,
                    )


def _lane_of(inst):
    from concourse.tile_sem_assignment import PROC_NAME_TO_IDX

    rev = {v: k for k, v in PROC_NAME_TO_IDX.items()}
    nm = rev.get(inst.bass_scheduled_proc, "")
    return int(nm[5:]) if nm.startswith("DMASW") else None


def build(queue_plan="auto"):
    import concourse.mybir as mybir
    from concourse import bacc, tile

    dt = mybir.dt
    nc = bacc.Bacc(
        "TRN2",
        target_bir_lowering=False,
        debug=False,
        enable_asserts=False,
        num_devices=NCORES,
        num_swdge_queues=2,
    )
    aps = {
        "x": nc.dram_tensor("x", [C, HW], dt.float32, kind="ExternalInput").ap(),
        "offset": nc.dram_tensor(
            "offset", [2 * K, HW], dt.float32, kind="ExternalInput"
        ).ap(),
        "w2": nc.dram_tensor(
            "w2", [C * K, O], dt.float32, kind="ExternalInput"
        ).ap(),
        "out": nc.dram_tensor(
            "out", [O, HW], dt.float32, kind="ExternalOutput"
        ).ap(),
    }
    if queue_plan == "auto":
        # pass 1: discover each SWDGE DMA's DMASW lane, then rebuild with a
        # lane-consistent queue assignment (lane%2, forced 0 on lanes that
        # host plain queue-0 dma_starts).
        rec = {"gather": [], "plain": []}
        with tile.TileContext(nc) as tc:
            _emit(tc, nc, aps, rec=rec, queue_plan=None)
        plain_lanes = {_lane_of(i) for i in rec["plain"]}
        plan = []
        for gi in rec["gather"]:
            lane = _lane_of(gi)
            q = 0 if (lane is None or lane in plain_lanes) else lane % 2
            plan.append(q)
        return build(plan)
    with tile.TileContext(nc) as tc:
        _emit(tc, nc, aps, queue_plan=queue_plan)
    nc.compile()
    return nc


def prep_in_maps(x, offset, weight):
    x = np.asarray(x, dtype=np.float32)
    offset = np.asarray(offset, dtype=np.float32)
    weight = np.asarray(weight, dtype=np.float32)
    w2 = np.ascontiguousarray(
        weight.reshape(O, C, K).transpose(2, 1, 0).reshape(C * K, O)
    )
    in_maps = []
    for b in range(NCORES):
        in_maps.append(
            {
                "x": np.ascontiguousarray(x[b].reshape(C, HW)),
                "offset": np.ascontiguousarray(offset[b].reshape(2 * K, HW)),
                "w2": w2,
            }
        )
    return in_maps


def run(x, offset, weight, trace=False, **kw):
    from concourse import bass_utils

    if "nc" not in _CACHE:
        _CACHE["nc"] = build()
    nc = _CACHE["nc"]
    res = bass_utils.run_bass_kernel_spmd(
        nc, prep_in_maps(x, offset, weight), core_ids=list(range(NCORES)),
        trace=trace, **kw,
    )
    out = np.stack([r["out"].reshape(O, H, W) for r in res.results])
    return out, res


def kernel(x, offset, weight):
    out, _ = run(x, offset, weight, trace=False)
    return out
